# revision 7
# baseline (speedup 1.0000x reference)
"""GCN-Attention kernel for Trainium2, data-parallel over 8 NeuronCores.

Reference computation (per image b of 64, category c of 100):
  full = concat(image_features, bbox)                    [N, 2052]
  x[b,c,:] = sum_{boxes n in bucket(b,c), slot<3} lin_w[slot]*full[n] + lin_b
  support  = x @ gc_w                                    [B, 100, 2048]
  gcn      = leaky_relu((X + adj) @ support + gc_b)
  out[b]   = global_features[b] @ gcn[b]                 [B, 2048]

Device mapping (per core, 8 images):
  phase 1: x = AT^T-scatter matmul (f32r, N=512 chunks) -> PE transpose
           (exact) -> x^T tiles [128,100] in SBUF
  phase 2: support chunk [100,512] = x^T (stationary) x gc_w (moving, f32r,
           resident in SBUF)
  phase 3: (X+adj)^T @ support + rank-2 bias fold (gc_b row + lin_b*colsum
           row), Lrelu on the scalar engine
  phase 4: attention row matmul, PSUM -> DRAM output

The scatter (occurrence slots within (image, category) buckets) is turned
into a small per-image weight matrix AT[box, cat] = lin_w[slot] on the host;
lin_b is folded into phase 3 via a host-computed rank-2 term.
"""
import numpy as np

import concourse.bacc as bacc
import concourse.mybir as mybir
import concourse.tile as tile
from concourse import bass_utils

B = 64
C = 100
LOOP = 3
FEAT = 2052
OUT = 2048
NCORES = 8
BPC = B // NCORES  # images per core

f32 = mybir.dt.float32
f32r = mybir.dt.float32r

_programs: dict[int, object] = {}
last_results = None  # BassKernelResults of the most recent run (for test harnesses)


def _occ_slots(key):
    """Occurrence index among equal-valued keys, stable order (matches jax ref)."""
    n = key.shape[0]
    order = np.argsort(key, kind="stable")
    sk = key[order]
    idx = np.arange(n)
    is_new = np.concatenate([[True], sk[1:] != sk[:-1]]) if n else np.zeros(0, bool)
    run_start = np.maximum.accumulate(np.where(is_new, idx, 0))
    pos = idx - run_start
    slots = np.zeros(n, np.int64)
    slots[order] = pos
    return slots


def _d_chunks():
    """Feature-dim chunks: 4x512 + 1x4 (2052 total)."""
    ch = [(i * 512, 512) for i in range(4)]
    ch.append((2048, FEAT - 2048))
    return ch


def _build_program(cap: int):
    """Emit the Bass/Tile program for `cap` (padded boxes per image)."""
    nkc = max(1, (cap + 127) // 128)  # box K-chunks per image
    kw_of = lambda kk: min(128, cap - kk * 128)

    nc = bacc.Bacc("TRN2", target_bir_lowering=False, debug=False,
                   num_devices=NCORES)

    full_d = nc.dram_tensor("full", [BPC * cap, FEAT], f32r, kind="ExternalInput").ap()
    at_d = nc.dram_tensor("at", [BPC * cap, C], f32r, kind="ExternalInput").ap()
    gcw_d = nc.dram_tensor("gcw", [FEAT, OUT], f32r, kind="ExternalInput").ap()
    adjT_d = nc.dram_tensor("adjT", [BPC, C, C], f32r, kind="ExternalInput").ap()
    adj2_d = nc.dram_tensor("adj2", [BPC, 2, C], f32r, kind="ExternalInput").ap()
    bias2_d = nc.dram_tensor("bias2", [2, OUT], f32r, kind="ExternalInput").ap()
    gT_d = nc.dram_tensor("gT", [C, BPC], f32r, kind="ExternalInput").ap()
    ident_d = nc.dram_tensor("ident", [C, C], f32r, kind="ExternalInput").ap()
    out_d = nc.dram_tensor("out", [BPC, OUT], f32, kind="ExternalOutput").ap()

    n_kt = (FEAT + 127) // 128  # 17 gc_w K-chunks (feature dim)
    dch = _d_chunks()

    with tile.TileContext(nc) as tc:
        with tc.tile_pool(name="const", bufs=1) as cpool, \
             tc.tile_pool(name="sb", bufs=1) as pool, \
             tc.tile_pool(name="ps", bufs=1, space="PSUM") as psp:

            # ---- resident constants ----
            gcw_sb = cpool.tile([128, n_kt * OUT], f32r, tag="gcw")
            for k in range(n_kt):
                kw = min(128, FEAT - k * 128)
                nc.sync.dma_start(
                    gcw_sb[0:kw, k * OUT:(k + 1) * OUT],
                    gcw_d[k * 128:k * 128 + kw, :],
                )
            bias2_sb = cpool.tile([2, OUT], f32r, tag="bias2")
            nc.sync.dma_start(bias2_sb[:], bias2_d[:])
            gT_sb = cpool.tile([C, BPC], f32r, tag="gT")
            nc.sync.dma_start(gT_sb[:], gT_d[:])
            ident_sb = cpool.tile([C, C], f32r, tag="ident")
            nc.sync.dma_start(ident_sb[:], ident_d[:])

            for b in range(BPC):
                # ---- per-image input tiles ----
                full_t, at_t = [], []
                for kk in range(nkc):
                    kw = kw_of(kk)
                    r0 = b * cap + kk * 128
                    ft = pool.tile([128, FEAT], f32r, tag="full", bufs=nkc)
                    nc.sync.dma_start(ft[0:kw, :], full_d[r0:r0 + kw, :])
                    at = pool.tile([128, C], f32r, tag="at", bufs=2 * nkc)
                    nc.sync.dma_start(at[0:kw, :], at_d[r0:r0 + kw, :])
                    full_t.append(ft)
                    at_t.append(at)
                adjT_sb = pool.tile([C, C], f32r, tag="adjT", bufs=2)
                nc.sync.dma_start(adjT_sb[:], adjT_d[b])
                adj2_sb = pool.tile([2, C], f32r, tag="adj2", bufs=2)
                nc.sync.dma_start(adj2_sb[:], adj2_d[b])

                # ---- phase 1: x chunks + transpose to x^T ----
                xT = [None] * n_kt
                for dof, dw in dch:
                    xp = psp.tile([C, dw], f32, tag="xps", bufs=2)
                    for kk in range(nkc):
                        kw = kw_of(kk)
                        nc.tensor.matmul(
                            xp[0:C, 0:dw],
                            at_t[kk][0:kw, 0:C],
                            full_t[kk][0:kw, dof:dof + dw],
                            start=(kk == 0), stop=(kk == nkc - 1),
                        )
                    xsb = pool.tile([C, dw], f32r, tag="x", bufs=3)
                    nc.vector.tensor_copy(xsb[:], xp[0:C, 0:dw])
                    for j in range((dw + 127) // 128):
                        w = min(128, dw - j * 128)
                        tp = psp.tile([128, C], f32r, tag="tps", bufs=2)
                        nc.tensor.transpose(
                            tp[0:w, 0:C],
                            xsb[0:C, j * 128:j * 128 + w],
                            ident_sb[0:C, 0:C],
                        )
                        xt = pool.tile([128, C], f32r, tag="xT", bufs=2 * n_kt)
                        nc.vector.tensor_copy(xt[0:w, :], tp[0:w, 0:C])
                        xT[(dof + j * 128) // 128] = (xt, w)

                # ---- phases 2-4, chunked over the output dim ----
                for nch in range(OUT // 512):
                    o0 = nch * 512
                    sp = psp.tile([C, 512], f32, tag="sps", bufs=2)
                    for k in range(n_kt):
                        xt, w = xT[k]
                        nc.tensor.matmul(
                            sp[0:C, 0:512],
                            xt[0:w, 0:C],
                            gcw_sb[0:w, k * OUT + o0:k * OUT + o0 + 512],
                            start=(k == 0), stop=(k == n_kt - 1),
                        )
                    ssb = pool.tile([C, 512], f32r, tag="sup", bufs=3)
                    nc.vector.tensor_copy(ssb[:], sp[0:C, 0:512])

                    gp = psp.tile([C, 512], f32, tag="gps", bufs=1)
                    nc.tensor.matmul(gp[0:C, 0:512], adjT_sb[0:C, 0:C],
                                     ssb[0:C, 0:512], start=True, stop=False)
                    nc.tensor.matmul(gp[0:C, 0:512], adj2_sb[0:2, 0:C],
                                     bias2_sb[0:2, o0:o0 + 512],
                                     start=False, stop=True)
                    gsb = pool.tile([C, 512], f32r, tag="gcn", bufs=3)
                    nc.scalar.activation(
                        gsb[:], gp[0:C, 0:512],
                        mybir.ActivationFunctionType.Lrelu, alpha=0.01,
                    )
                    op = psp.tile([1, 512], f32, tag="ops", bufs=1)
                    nc.tensor.matmul(op[0:1, 0:512], gT_sb[0:C, b:b + 1],
                                     gsb[0:C, 0:512], start=True, stop=True)
                    ostage = pool.tile([1, 512], f32, tag="ostage", bufs=2)
                    nc.vector.tensor_copy(ostage[0:1, 0:512], op[0:1, 0:512])
                    nc.sync.dma_start(out_d[b:b + 1, o0:o0 + 512], ostage[0:1, 0:512])

    nc.compile()
    return nc


def _get_program(cap: int):
    if cap not in _programs:
        _programs[cap] = _build_program(cap)
    return _programs[cap]


def kernel(**inputs) -> np.ndarray:
    global last_results
    imf = np.asarray(inputs["image_features"], np.float32)
    bbox = np.asarray(inputs["bbox_list"], np.float32)
    gf = np.asarray(inputs["global_features"], np.float32)
    adj = np.asarray(inputs["adj"], np.float32)
    X = np.asarray(inputs["X"], np.float32)
    lin_w = np.asarray(inputs["lin_w"], np.float32)
    lin_b = np.float32(np.asarray(inputs["lin_b"]))
    gc_w = np.ascontiguousarray(np.asarray(inputs["gc_w"], np.float32))
    gc_b = np.asarray(inputs["gc_b"], np.float32)
    label = np.asarray(inputs["label_list"]).astype(np.int64)
    batch = np.asarray(inputs["batch"]).astype(np.int64)

    n = imf.shape[0]
    full = np.concatenate([imf, bbox], axis=1)

    # scatter bookkeeping, matching jax semantics: slots by stable order of
    # key=batch*C+(label-1); negative cats wrap, slot>=LOOP / far-oob dropped
    cat = label - 1
    key = batch * C + cat
    slots = _occ_slots(key)
    valid = (slots < LOOP) & (cat >= -C) & (cat < C)
    wvals = np.where(valid, lin_w[np.clip(slots, 0, LOOP - 1)], 0.0).astype(np.float32)
    cidx = np.mod(cat, C).astype(np.int64)

    # boxes must be grouped by image for per-image slicing
    if np.any(batch[1:] < batch[:-1]):
        perm = np.argsort(batch, kind="stable")
        batch, full, wvals, cidx, valid = (
            batch[perm], full[perm], wvals[perm], cidx[perm], valid[perm])

    lo = np.searchsorted(batch, np.arange(B))
    hi = np.searchsorted(batch, np.arange(B), side="right")
    counts = hi - lo
    cap = max(int(counts.max()) if n else 1, 1)

    newadj = X[None, :, :] + adj
    adjT = np.ascontiguousarray(newadj.transpose(0, 2, 1)).astype(np.float32)
    rowsum = newadj.sum(axis=2).astype(np.float32)
    bias2 = np.stack([gc_b, lin_b * gc_w.sum(axis=0)]).astype(np.float32)
    ident = np.eye(C, dtype=np.float32)

    in_maps = []
    for core in range(NCORES):
        imgs = slice(core * BPC, (core + 1) * BPC)
        fullp = np.zeros((BPC * cap, FEAT), np.float32)
        atp = np.zeros((BPC * cap, C), np.float32)
        for j, bimg in enumerate(range(core * BPC, (core + 1) * BPC)):
            l, h = int(lo[bimg]), int(hi[bimg])
            m = h - l
            if m == 0:
                continue
            fullp[j * cap:j * cap + m] = full[l:h]
            v = valid[l:h]
            rows = j * cap + np.arange(m)[v]
            atp[rows, cidx[l:h][v]] = wvals[l:h][v]
        adj2 = np.stack(
            [np.ones((BPC, C), np.float32), rowsum[imgs]], axis=1
        ).astype(np.float32)
        in_maps.append(dict(
            full=fullp, at=atp, gcw=gc_w, adjT=adjT[imgs], adj2=adj2,
            bias2=bias2, gT=np.ascontiguousarray(gf[imgs].T).astype(np.float32),
            ident=ident,
        ))

    nc = _get_program(cap)
    res = bass_utils.run_bass_kernel_spmd(nc, in_maps, core_ids=list(range(NCORES)))
    last_results = res
    return np.concatenate([res.results[i]["out"] for i in range(NCORES)], axis=0)


# revision 8
# speedup vs baseline: 1.2893x; 1.2893x over previous
"""GCN-Attention kernel for Trainium2, data-parallel over 8 NeuronCores.

Reference computation (per image b of 64, category c of 100):
  full = concat(image_features, bbox)                    [N, 2052]
  x[b,c,:] = sum_{boxes n in bucket(b,c), slot<3} lin_w[slot]*full[n] + lin_b
  support  = x @ gc_w                                    [B, 100, 2048]
  gcn      = leaky_relu((X + adj) @ support + gc_b)
  out[b]   = global_features[b] @ gcn[b]                 [B, 2048]

Device mapping (per core, 8 images), bf16 matmuls with fp32 PSUM accumulate:
  phase 1: x^T tiles [128,100] built directly: for each 128-wide feature
           chunk m, out = full_chunk^T @ AT (boxes contracted), where
           AT[box, cat] = lin_w[slot] is a host-built scatter-weight matrix.
  phase 2: support chunk [100,512] = x^T_k (stationary) x gc_w_k (moving),
           gc_w resident in SBUF, accumulated over 17 feature K-chunks.
  phase 3: one K=102 matmul: [adjT | ones | rowsum]^T @ [support; gc_b;
           lin_b*colsum(gc_w)] folds both bias terms, Lrelu on scalar engine.
  phase 4: attention row matmul, then DVE copy + DMA out (fp32).

An f32r (tf32-like) variant is kept behind KERNEL_PRECISE=1 for tighter
accuracy at ~1.5x the runtime.
"""
import os

import ml_dtypes
import numpy as np

import concourse.bacc as bacc
import concourse.mybir as mybir
import concourse.tile as tile
from concourse import bass_utils

B = 64
C = 100
LOOP = 3
FEAT = 2052
OUT = 2048
NCORES = 8
BPC = B // NCORES  # images per core

f32 = mybir.dt.float32
f32r = mybir.dt.float32r
bf16 = mybir.dt.bfloat16
np_bf16 = ml_dtypes.bfloat16

_programs: dict = {}
last_results = None  # BassKernelResults of the most recent run (for harnesses)


def _occ_slots(key):
    """Occurrence index among equal-valued keys, stable order (matches jax ref)."""
    n = key.shape[0]
    order = np.argsort(key, kind="stable")
    sk = key[order]
    idx = np.arange(n)
    is_new = np.concatenate([[True], sk[1:] != sk[:-1]]) if n else np.zeros(0, bool)
    run_start = np.maximum.accumulate(np.where(is_new, idx, 0))
    pos = idx - run_start
    slots = np.zeros(n, np.int64)
    slots[order] = pos
    return slots


def _build_bf16(cap: int):
    """bf16 pipeline: direct x^T (no transposes), gc_w resident bf16."""
    nkc = max(1, (cap + 127) // 128)
    kw_of = lambda kk: min(128, cap - kk * 128)
    n_kt = (FEAT + 127) // 128  # 17 feature chunks
    mw_of = lambda m: min(128, FEAT - m * 128)

    nc = bacc.Bacc("TRN2", target_bir_lowering=False, debug=False,
                   num_devices=NCORES)

    full_d = nc.dram_tensor("full", [BPC * cap, FEAT], bf16, kind="ExternalInput").ap()
    at_d = nc.dram_tensor("at", [BPC * cap, C], bf16, kind="ExternalInput").ap()
    gcw_d = nc.dram_tensor("gcw", [FEAT, OUT], bf16, kind="ExternalInput").ap()
    adjT_d = nc.dram_tensor("adjT", [BPC, C + 2, C], bf16, kind="ExternalInput").ap()
    bias2_d = nc.dram_tensor("bias2", [2, OUT], bf16, kind="ExternalInput").ap()
    gT_d = nc.dram_tensor("gT", [C, BPC], bf16, kind="ExternalInput").ap()
    out_d = nc.dram_tensor("out", [BPC, OUT], f32, kind="ExternalOutput").ap()

    with tile.TileContext(nc) as tc:
        with tc.tile_pool(name="const", bufs=1) as cpool, \
             tc.tile_pool(name="sb", bufs=1) as pool, \
             tc.tile_pool(name="ps", bufs=1, space="PSUM") as psp:

            # gc_w resident, streamed on the gpsimd DMA queue so per-image
            # input loads on the sync queue are not serialized behind it
            gcw_sb = cpool.tile([128, n_kt * OUT], bf16, tag="gcw")
            for k in range(n_kt):
                kw = mw_of(k)
                nc.gpsimd.dma_start(
                    gcw_sb[0:kw, k * OUT:(k + 1) * OUT],
                    gcw_d[k * 128:k * 128 + kw, :],
                )
            gT_sb = cpool.tile([C, BPC], bf16, tag="gT")
            nc.sync.dma_start(gT_sb[:], gT_d[:])

            for b in range(BPC):
                full_t, at_t = [], []
                for kk in range(nkc):
                    kw = kw_of(kk)
                    r0 = b * cap + kk * 128
                    ft = pool.tile([128, FEAT], bf16, tag="full", bufs=2 * nkc)
                    nc.sync.dma_start(ft[0:kw, :], full_d[r0:r0 + kw, :])
                    at = pool.tile([128, C], bf16, tag="at", bufs=2 * nkc)
                    nc.sync.dma_start(at[0:kw, :], at_d[r0:r0 + kw, :])
                    full_t.append(ft)
                    at_t.append(at)
                adjT_sb = pool.tile([C + 2, C], bf16, tag="adjT", bufs=2)
                nc.sync.dma_start(adjT_sb[:], adjT_d[b])

                # ---- phase 1: x^T chunks straight from boxes ----
                xT = [None] * n_kt
                for m in range(n_kt):
                    mw = mw_of(m)
                    xp = psp.tile([128, C], f32, tag="xps", bufs=3)
                    for kk in range(nkc):
                        kw = kw_of(kk)
                        nc.tensor.matmul(
                            xp[0:mw, 0:C],
                            full_t[kk][0:kw, m * 128:m * 128 + mw],
                            at_t[kk][0:kw, 0:C],
                            start=(kk == 0), stop=(kk == nkc - 1),
                        )
                    xt = pool.tile([128, C], bf16, tag="xT", bufs=2 * n_kt)
                    nc.vector.tensor_copy(xt[0:mw, :], xp[0:mw, 0:C])
                    xT[m] = (xt, mw)

                # ---- phases 2-4 per 512-wide output chunk ----
                for nch in range(OUT // 512):
                    o0 = nch * 512
                    sp = psp.tile([C, 512], f32, tag="sps", bufs=2)
                    for k in range(n_kt):
                        xt, mw = xT[k]
                        nc.tensor.matmul(
                            sp[0:C, 0:512],
                            xt[0:mw, 0:C],
                            gcw_sb[0:mw, k * OUT + o0:k * OUT + o0 + 512],
                            start=(k == 0), stop=(k == n_kt - 1),
                        )
                    ssb = pool.tile([C + 2, 512], bf16, tag="sup", bufs=4)
                    nc.vector.tensor_copy(ssb[0:C, :], sp[0:C, 0:512])
                    nc.sync.dma_start(ssb[C:C + 2, :], bias2_d[0:2, o0:o0 + 512])

                    gp = psp.tile([C, 512], f32, tag="gps", bufs=1)
                    nc.tensor.matmul(gp[0:C, 0:512], adjT_sb[0:C + 2, 0:C],
                                     ssb[0:C + 2, 0:512], start=True, stop=True)
                    gsb = pool.tile([C, 512], bf16, tag="gcn", bufs=3)
                    nc.scalar.activation(
                        gsb[:], gp[0:C, 0:512],
                        mybir.ActivationFunctionType.Lrelu, alpha=0.01,
                    )
                    op = psp.tile([1, 512], f32, tag="ops", bufs=1)
                    nc.tensor.matmul(op[0:1, 0:512], gT_sb[0:C, b:b + 1],
                                     gsb[0:C, 0:512], start=True, stop=True)
                    ostage = pool.tile([1, 512], f32, tag="ostage", bufs=2)
                    nc.vector.tensor_copy(ostage[0:1, 0:512], op[0:1, 0:512])
                    nc.sync.dma_start(out_d[b:b + 1, o0:o0 + 512],
                                      ostage[0:1, 0:512])

    nc.compile()
    return nc


def _d_chunks():
    ch = [(i * 512, 512) for i in range(4)]
    ch.append((2048, FEAT - 2048))
    return ch


def _build_f32r(cap: int):
    """f32r pipeline (route B: x then PE transpose); ~2e-4 rel err."""
    nkc = max(1, (cap + 127) // 128)
    kw_of = lambda kk: min(128, cap - kk * 128)
    n_kt = (FEAT + 127) // 128

    nc = bacc.Bacc("TRN2", target_bir_lowering=False, debug=False,
                   num_devices=NCORES)

    full_d = nc.dram_tensor("full", [BPC * cap, FEAT], f32r, kind="ExternalInput").ap()
    at_d = nc.dram_tensor("at", [BPC * cap, C], f32r, kind="ExternalInput").ap()
    gcw_d = nc.dram_tensor("gcw", [FEAT, OUT], f32r, kind="ExternalInput").ap()
    adjT_d = nc.dram_tensor("adjT", [BPC, C + 2, C], f32r, kind="ExternalInput").ap()
    bias2_d = nc.dram_tensor("bias2", [2, OUT], f32r, kind="ExternalInput").ap()
    gT_d = nc.dram_tensor("gT", [C, BPC], f32r, kind="ExternalInput").ap()
    ident_d = nc.dram_tensor("ident", [C, C], f32r, kind="ExternalInput").ap()
    out_d = nc.dram_tensor("out", [BPC, OUT], f32, kind="ExternalOutput").ap()

    dch = _d_chunks()

    with tile.TileContext(nc) as tc:
        with tc.tile_pool(name="const", bufs=1) as cpool, \
             tc.tile_pool(name="sb", bufs=1) as pool, \
             tc.tile_pool(name="ps", bufs=1, space="PSUM") as psp:

            gcw_sb = cpool.tile([128, n_kt * OUT], f32r, tag="gcw")
            for k in range(n_kt):
                kw = min(128, FEAT - k * 128)
                nc.gpsimd.dma_start(
                    gcw_sb[0:kw, k * OUT:(k + 1) * OUT],
                    gcw_d[k * 128:k * 128 + kw, :],
                )
            gT_sb = cpool.tile([C, BPC], f32r, tag="gT")
            nc.sync.dma_start(gT_sb[:], gT_d[:])
            ident_sb = cpool.tile([C, C], f32r, tag="ident")
            nc.sync.dma_start(ident_sb[:], ident_d[:])

            for b in range(BPC):
                full_t, at_t = [], []
                for kk in range(nkc):
                    kw = kw_of(kk)
                    r0 = b * cap + kk * 128
                    ft = pool.tile([128, FEAT], f32r, tag="full", bufs=nkc)
                    nc.sync.dma_start(ft[0:kw, :], full_d[r0:r0 + kw, :])
                    at = pool.tile([128, C], f32r, tag="at", bufs=2 * nkc)
                    nc.sync.dma_start(at[0:kw, :], at_d[r0:r0 + kw, :])
                    full_t.append(ft)
                    at_t.append(at)
                adjT_sb = pool.tile([C + 2, C], f32r, tag="adjT", bufs=2)
                nc.sync.dma_start(adjT_sb[:], adjT_d[b])

                xT = [None] * n_kt
                for dof, dw in dch:
                    xp = psp.tile([C, dw], f32, tag="xps", bufs=2)
                    for kk in range(nkc):
                        kw = kw_of(kk)
                        nc.tensor.matmul(
                            xp[0:C, 0:dw],
                            at_t[kk][0:kw, 0:C],
                            full_t[kk][0:kw, dof:dof + dw],
                            start=(kk == 0), stop=(kk == nkc - 1),
                        )
                    xsb = pool.tile([C, dw], f32r, tag="x", bufs=3)
                    nc.vector.tensor_copy(xsb[:], xp[0:C, 0:dw])
                    for j in range((dw + 127) // 128):
                        w = min(128, dw - j * 128)
                        tp = psp.tile([128, C], f32r, tag="tps", bufs=2)
                        nc.tensor.transpose(
                            tp[0:w, 0:C],
                            xsb[0:C, j * 128:j * 128 + w],
                            ident_sb[0:C, 0:C],
                        )
                        xt = pool.tile([128, C], f32r, tag="xT", bufs=2 * n_kt)
                        nc.vector.tensor_copy(xt[0:w, :], tp[0:w, 0:C])
                        xT[(dof + j * 128) // 128] = (xt, w)

                for nch in range(OUT // 512):
                    o0 = nch * 512
                    sp = psp.tile([C, 512], f32, tag="sps", bufs=1)
                    for k in range(n_kt):
                        xt, w = xT[k]
                        nc.tensor.matmul(
                            sp[0:C, 0:512],
                            xt[0:w, 0:C],
                            gcw_sb[0:w, k * OUT + o0:k * OUT + o0 + 512],
                            start=(k == 0), stop=(k == n_kt - 1),
                        )
                    ssb = pool.tile([C + 2, 512], f32r, tag="sup", bufs=3)
                    nc.vector.tensor_copy(ssb[0:C, :], sp[0:C, 0:512])
                    nc.sync.dma_start(ssb[C:C + 2, :], bias2_d[0:2, o0:o0 + 512])

                    gp = psp.tile([C, 512], f32, tag="gps", bufs=1)
                    nc.tensor.matmul(gp[0:C, 0:512], adjT_sb[0:C + 2, 0:C],
                                     ssb[0:C + 2, 0:512], start=True, stop=True)
                    gsb = pool.tile([C, 512], f32r, tag="gcn", bufs=3)
                    nc.scalar.activation(
                        gsb[:], gp[0:C, 0:512],
                        mybir.ActivationFunctionType.Lrelu, alpha=0.01,
                    )
                    op = psp.tile([1, 512], f32, tag="ops", bufs=1)
                    nc.tensor.matmul(op[0:1, 0:512], gT_sb[0:C, b:b + 1],
                                     gsb[0:C, 0:512], start=True, stop=True)
                    ostage = pool.tile([1, 512], f32, tag="ostage", bufs=2)
                    nc.vector.tensor_copy(ostage[0:1, 0:512], op[0:1, 0:512])
                    nc.sync.dma_start(out_d[b:b + 1, o0:o0 + 512],
                                      ostage[0:1, 0:512])

    nc.compile()
    return nc


def _get_program(cap: int, precise: bool):
    key = (cap, precise)
    if key not in _programs:
        _programs[key] = _build_f32r(cap) if precise else _build_bf16(cap)
    return _programs[key]


def kernel(**inputs) -> np.ndarray:
    global last_results
    precise = os.environ.get("KERNEL_PRECISE", "0") == "1"
    mmdt = np.float32 if precise else np_bf16

    imf = np.asarray(inputs["image_features"], np.float32)
    bbox = np.asarray(inputs["bbox_list"], np.float32)
    gf = np.asarray(inputs["global_features"], np.float32)
    adj = np.asarray(inputs["adj"], np.float32)
    X = np.asarray(inputs["X"], np.float32)
    lin_w = np.asarray(inputs["lin_w"], np.float32)
    lin_b = np.float32(np.asarray(inputs["lin_b"]))
    gc_w = np.ascontiguousarray(np.asarray(inputs["gc_w"], np.float32))
    gc_b = np.asarray(inputs["gc_b"], np.float32)
    label = np.asarray(inputs["label_list"]).astype(np.int64)
    batch = np.asarray(inputs["batch"]).astype(np.int64)

    n = imf.shape[0]
    full = np.concatenate([imf, bbox], axis=1)

    # scatter bookkeeping, matching jax semantics: slots by stable order of
    # key=batch*C+(label-1); negative cats wrap, slot>=LOOP / far-oob dropped
    cat = label - 1
    key = batch * C + cat
    slots = _occ_slots(key)
    valid = (slots < LOOP) & (cat >= -C) & (cat < C)
    wvals = np.where(valid, lin_w[np.clip(slots, 0, LOOP - 1)], 0.0).astype(np.float32)
    cidx = np.mod(cat, C).astype(np.int64)

    # boxes must be grouped by image for per-image slicing
    if np.any(batch[1:] < batch[:-1]):
        perm = np.argsort(batch, kind="stable")
        batch, full, wvals, cidx, valid = (
            batch[perm], full[perm], wvals[perm], cidx[perm], valid[perm])

    lo = np.searchsorted(batch, np.arange(B))
    hi = np.searchsorted(batch, np.arange(B), side="right")
    counts = hi - lo
    cap = max(int(counts.max()) if n else 1, 1)

    newadj = X[None, :, :] + adj                               # [B, C, C]
    rowsum = newadj.sum(axis=2).astype(np.float32)             # [B, C]
    # [B, C+2, C]: rows 0..99 = newadj^T, row 100 = ones (gc_b), 101 = rowsum
    adjTa = np.empty((B, C + 2, C), np.float32)
    adjTa[:, 0:C, :] = newadj.transpose(0, 2, 1)
    adjTa[:, C, :] = 1.0
    adjTa[:, C + 1, :] = rowsum
    bias2 = np.stack([gc_b, lin_b * gc_w.sum(axis=0)]).astype(np.float32)
    ident = np.eye(C, dtype=np.float32)

    in_maps = []
    for core in range(NCORES):
        imgs = slice(core * BPC, (core + 1) * BPC)
        fullp = np.zeros((BPC * cap, FEAT), np.float32)
        atp = np.zeros((BPC * cap, C), np.float32)
        for j, bimg in enumerate(range(core * BPC, (core + 1) * BPC)):
            l, h = int(lo[bimg]), int(hi[bimg])
            m = h - l
            if m == 0:
                continue
            fullp[j * cap:j * cap + m] = full[l:h]
            v = valid[l:h]
            rows = j * cap + np.arange(m)[v]
            atp[rows, cidx[l:h][v]] = wvals[l:h][v]
        im = dict(
            full=fullp.astype(mmdt), at=atp.astype(mmdt), gcw=gc_w.astype(mmdt),
            adjT=adjTa[imgs].astype(mmdt), bias2=bias2.astype(mmdt),
            gT=np.ascontiguousarray(gf[imgs].T).astype(mmdt),
        )
        if precise:
            im["ident"] = ident
        in_maps.append(im)

    nc = _get_program(cap, precise)
    res = bass_utils.run_bass_kernel_spmd(nc, in_maps, core_ids=list(range(NCORES)))
    last_results = res
    return np.concatenate([res.results[i]["out"] for i in range(NCORES)], axis=0)
